# revision 26
# baseline (speedup 1.0000x reference)
"""Trainium2 Bass kernel for nn_CausalTemporalAttention.

Reference semantics (B == L == H == 8 required by the module's broadcast quirks):
  qkv = x @ w_qkv.T + b_qkv ; split q,k,v -> [B,L,H,S,d]
  scores[b,l,h,s,t] = q.k/sqrt(d) ; masked to -1e9 where h > l
  z = scores * decay_params[b,l,h] ; attn = softmax over l (the layer axis)
  out[b,l,h,s,:] = attn @ v ; swap (l,h) ; row-major reshape to [B*H, S, E]
  y = out @ w_out.T + b_out ; reshape [B,L,S,E]

Sharding: data-parallel over batch B across 8 cores (core i handles b=i).

Design:
  - q/k projections run in fp8e4 DoubleRow perf mode (K=256 per pass, 2x PE
    throughput); numerically validated at rel-err ~1.3e-2 vs the 2e-2 gate.
  - All other matmuls and on-chip intermediates are fp16 (same PE/DVE speed
    as bf16, 8x finer mantissa) so the fp8 stage gets the whole error budget.
  - Host permutes tokens within each layer (s~ = (s%8)*32 + s//8) so the
    attn@v output lands in the out-projection's scrambled (j, si) order and
    the PSUM->SBUF scatter becomes a contiguous copy.
  - Head 7 sees a single unmasked layer, so its attention weights are exactly
    1: attn@v collapses to a ones-matmul row sum and skips softmax entirely.
  - Out-projection only computes the nonzero rows s' >= 32h of each head's
    output; the zero rows are DMA-filled from a zero tile early on.
  - Softmax denominator: exp pair-sums on Pool, short reduce + one
    reciprocal_approx_fast (fp32 custom DVE op) on DVE, dtype casts on ACT.
  - x is loaded via two DMA queues in consumption order (the front is
    DMA-arrival bound otherwise); bulk weight loads sit on queues with no
    compute behind them.
"""

import os
import sys

import numpy as np
import ml_dtypes

if "/opt/trn_rl_repo" not in sys.path:
    sys.path.insert(0, "/opt/trn_rl_repo")

B, L, S, E = 8, 8, 256, 1024
H, d = 8, E // 8
T = L * S            # 2048 tokens per batch element
NE = E // 128        # 8 e-chunks
NP = NE // 2         # 4 fp8 DoubleRow e-chunk pairs
F = 3 * E

# token permutation within each layer: position p holds old token (p%32)*8+p//32
# so attn@v's moving axis comes out in the out-projection's (j=s%8, si=s//8)
# order and the gt scatter is contiguous.
PERM = np.array([(p % 32) * 8 + p // 32 for p in range(S)], dtype=np.int64)

# (l, h) pairs with h <= l, l-major so v-proj copies can batch whole h-groups
VBLK = {(l, h): l * (l + 1) // 2 + h for l in range(L) for h in range(l + 1)}
NVB = L * (L + 1) // 2  # 36 blocks

_BUILD_CACHE = {}


def _build(with_bias):
    import concourse.bass as bass
    import concourse.tile as tile
    import concourse.mybir as mybir
    from concourse import bacc
    from contextlib import ExitStack

    dt = mybir.dt
    AF = mybir.ActivationFunctionType
    DR = mybir.MatmulPerfMode.DoubleRow

    nc = bacc.Bacc("TRN2", target_bir_lowering=False, debug=False, num_devices=8)

    # fp8 x for q/k projection, quarter-major so one DMA lands one quarter with
    # 1KB-contiguous partition lines: [pair, q, p, kk, 512]
    x8_d = nc.dram_tensor("x8", [NP, 4, 128, 2, 512], dt.float8e4, kind="ExternalInput").ap()
    # fp16 x for the v projection (stationary side): [E, T]
    xT_d = nc.dram_tensor("xT", [E, T], dt.float16, kind="ExternalInput").ap()
    # q/k weights pre-packed as [part, head, p, (pair, kk, m)] so each
    # (part, head) is one contiguous [128, E] fp8 tile whose [:, pair, :, :]
    # slice is the DoubleRow stationary operand.
    wqk_d = nc.dram_tensor("wqk8", [2, H, 128, E], dt.float8e4, kind="ExternalInput").ap()
    wv_d = nc.dram_tensor("wvT", [E, E], dt.float16, kind="ExternalInput").ap()
    wo_d = nc.dram_tensor("woutT", [E, E], dt.float16, kind="ExternalInput").ap()
    dec_d = nc.dram_tensor("decay", [128, L * H], dt.float32, kind="ExternalInput").ap()
    if with_bias:
        bq_d = nc.dram_tensor("bqkv", [1, F], dt.float16, kind="ExternalInput").ap()
        bo_d = nc.dram_tensor("bout", [1, E], dt.float16, kind="ExternalInput").ap()
        bor_d = nc.dram_tensor("bout_row", [128, E], dt.float16, kind="ExternalInput").ap()
    y_d = nc.dram_tensor("y", [H, S, E], dt.float16, kind="ExternalOutput").ap()

    with ExitStack() as ctx:
        ctx.enter_context(
            nc.allow_low_precision(
                reason="fp8 q/k projection + fp16 softmax intermediates; "
                       "end-to-end error ~1.3e-2 of scale vs 2e-2 gate"
            )
        )
        tc = ctx.enter_context(tile.TileContext(nc))

        consts = ctx.enter_context(tc.tile_pool(name="consts", bufs=1))
        x8_sb = [consts.tile([128, 4, 2, 512], dt.float8e4, name=f"x8_{p}") for p in range(NP)]
        xT_sb = [consts.tile([128, T], dt.float16, name=f"xT{e}") for e in range(NE)]
        wqk_sb = {
            (part, h): consts.tile([128, NP, 2, 128], dt.float8e4, name=f"w{part}{h}")
            for part in ("q", "k")
            for h in range(H)
        }
        wv_sb = [consts.tile([128, E], dt.float16, name=f"wv{e}") for e in range(NE)]
        wo_sb = [consts.tile([128, E], dt.float16, name=f"wo{e}") for e in range(NE)]
        dec_sb = consts.tile([128, L * H], dt.float32)
        v_sb = consts.tile([128, NVB, 2, d], dt.float16)
        zrow_sb = consts.tile([128, 512], dt.float16)
        ones_t = consts.tile([128, S], dt.float16, name="ones_t")  # head-7 attn==1
        # all heads' nonzero out-proj rows packed into one [128, j, 1152] GT
        # so row-blocks can span head boundaries: 9 matmul blocks instead of 12
        gtm_sb = consts.tile([128, NE, 1152], dt.float16, name="gtm")

        if with_bias:
            bq_sb = consts.tile([1, F], dt.float16)
            bo_sb = consts.tile([1, E], dt.float16)
            ones_sb = consts.tile([1, 512], dt.float16)
            borow_sb = consts.tile([128, E], dt.float16)
            nc.gpsimd.dma_start(out=bq_sb, in_=bq_d)
            nc.gpsimd.dma_start(out=bo_sb, in_=bo_d)
            nc.gpsimd.dma_start(out=borow_sb, in_=bor_d)
            nc.gpsimd.memset(ones_sb, 1.0)

        # warm-up tiles first on the idle Pool engine so the PE clock-gate
        # opener isn't stuck behind DVE/ACT work
        warm_w = consts.tile([128, 128], dt.bfloat16, name="warm_w")
        warm_x = consts.tile([128, 512], dt.bfloat16, name="warm_x")
        nc.gpsimd.memset(warm_w, 0.0)
        nc.gpsimd.memset(warm_x, 0.0)
        nc.gpsimd.memset(zrow_sb, 0.0)
        nc.gpsimd.memset(ones_t, 1.0)
        nc.gpsimd.dma_start(out=dec_sb, in_=dec_d)

        # ---- DMA issue plan. Each dma_start lands on ONE ~25-40GB/s HW ring
        # (8 rings per issuing engine); the front consumes ~6MB of x in ~25us,
        # which saturates one queue's rings, so x is split across two queues in
        # exact consumption order. Queues that later run compute carry no DMA
        # tail (the bf16 baseline stalled head-0's exps behind weight DMAs).
        #   SP(sync): wqk8 h0 -> {x8 + xT16 lo-chunks per quarter} -> wqk8 h1
        #             -> wo16 -> zero-row y fills
        #   ACT:      {xT16 hi-chunks q0, wv g0, hi q1, wv g1, hi q2, hi q3}
        #             -> wqk8 h2..7   (all retired long before the first exp)
        def _wqk_dma(eng, pi, part, h):
            for half in range(2):
                eng.dma_start(
                    out=wqk_sb[(part, h)].rearrange("p a b c -> p (a b c)")[:, half * 512:(half + 1) * 512],
                    in_=wqk_d[pi, h, :, half * 512:(half + 1) * 512],
                )

        def _xT_dma(eng, e, q):
            eng.dma_start(
                out=xT_sb[e][:, q * 512:(q + 1) * 512],
                in_=xT_d[e * 128:(e + 1) * 128, q * 512:(q + 1) * 512],
            )

        # The first quarter's burst (wqk h0 + x8 q0 + xT q0, ~1.5MB) gates the
        # whole front; spread it over three queues' ring groups so the
        # transfers run in parallel instead of serializing on one queue.
        def _x8_halves(eng, p, q):
            for kk in range(2):
                eng.dma_start(out=x8_sb[p][:, q, kk, :], in_=x8_d[p, q, :, kk, :])

        _wqk_dma(nc.sync, 0, "q", 0)
        _x8_halves(nc.sync, 0, 0)
        _wqk_dma(nc.sync, 1, "k", 0)
        for e in range(2):
            _xT_dma(nc.sync, e, 0)
        for pi, part in ((0, "q"), (1, "k")):
            _wqk_dma(nc.sync, pi, part, 1)
        for q in range(1, 4):
            for p in range(NP):
                nc.sync.dma_start(out=x8_sb[p][:, q, :, :], in_=x8_d[p, q])
            for e in range(4):
                _xT_dma(nc.sync, e, q)
        # ACT queue carries only x chunks (done ~13us) so head-0's exps are
        # never stuck behind a DMA-issue tail; wv rides the Pool queue whose
        # compute (pair-sums) starts even later.
        _x8_halves(nc.scalar, 1, 0)
        _x8_halves(nc.scalar, 2, 0)
        for q in range(4):
            for e in range(4, NE):
                _xT_dma(nc.scalar, e, q)
            if q == 1:  # wv cols 512+ first needed by v_proj l=4 (~quarter 2)
                for e in range(NE):
                    nc.scalar.dma_start(
                        out=wv_sb[e][:, 512:], in_=wv_d[e * 128:(e + 1) * 128, 512:]
                    )
        _x8_halves(nc.gpsimd, 3, 0)
        for e in range(2, 4):
            _xT_dma(nc.gpsimd, e, 0)
        for e in range(NE):
            nc.gpsimd.dma_start(
                out=wv_sb[e][:, :512], in_=wv_d[e * 128:(e + 1) * 128, :512]
            )
        # late-needed weights + zero-row y fills on the sync tail, in
        # consumption order (h2.. weights first, zero fills last)
        for h in range(2, H):
            for pi, part in ((0, "q"), (1, "k")):
                _wqk_dma(nc.sync, pi, part, h)
        for e in range(NE):
            for half in range(2):
                nc.sync.dma_start(
                    out=wo_sb[e][:, half * 512:(half + 1) * 512],
                    in_=wo_d[e * 128:(e + 1) * 128, half * 512:(half + 1) * 512],
                )
        zsrc = borow_sb if with_bias else None
        for h in range(1, H):
            r = 32 * h
            for (r0, r1) in ((0, min(r, 128)), (128, r)):
                if r1 <= r0:
                    continue
                for ng in range(2):
                    src = (zsrc[:r1 - r0, ng * 512:(ng + 1) * 512] if with_bias
                           else zrow_sb[:r1 - r0, :])
                    nc.sync.dma_start(
                        out=y_d[h, r0:r1, ng * 512:(ng + 1) * 512], in_=src
                    )

        mm_ps = ctx.enter_context(tc.tile_pool(name="mm_ps", bufs=4, space="PSUM"))
        sc_ps = ctx.enter_context(tc.tile_pool(name="sc_ps", bufs=2, space="PSUM"))
        o2_ps = ctx.enter_context(tc.tile_pool(name="o2_ps", bufs=2, space="PSUM"))

        qk_pool = ctx.enter_context(tc.tile_pool(name="qk", bufs=3))
        exp_pool = ctx.enter_context(tc.tile_pool(name="expp", bufs=2))
        sm_pool = ctx.enter_context(tc.tile_pool(name="smp", bufs=2))
        at_pool = ctx.enter_context(tc.tile_pool(name="atp", bufs=3))
        out_pool = ctx.enter_context(tc.tile_pool(name="outp", bufs=3))

        def v_proj(tts, gs=(0, 1)):
            # v projection (natural [token, dd] layout): stationary xT chunk,
            # moving wv columns. Only heads h <= l are ever computed; copies
            # batch all h-blocks of one PSUM group (v_sb is l-major).
            for tt in tts:
                l = tt // 2
                ncols = 128 * (l + 1)
                for g in range((ncols + 511) // 512):
                    if g not in gs:
                        continue
                    n_g = min(512, ncols - 512 * g)
                    p_v = mm_ps.tile([128, n_g], dt.float32, tag="mm", name="p_v")
                    for e in range(NE):
                        nc.tensor.matmul(
                            p_v,
                            lhsT=xT_sb[e][:, tt * 128:(tt + 1) * 128],
                            rhs=wv_sb[e][:, 512 * g: 512 * g + n_g],
                            start=(e == 0),
                            stop=(e == NE - 1) and not with_bias,
                        )
                    if with_bias:
                        nc.tensor.matmul(
                            p_v,
                            lhsT=ones_sb[:, :128],
                            rhs=bq_sb[:, 2 * E + 512 * g: 2 * E + 512 * g + n_g],
                            start=False,
                            stop=True,
                        )
                    vb = VBLK[(l, 4 * g)]
                    nb = n_g // 128
                    nc.vector.tensor_copy(
                        out=v_sb[:, vb:vb + nb, tt % 2, :],
                        in_=p_v.rearrange("p (b m) -> p b m", b=nb),
                    )

        # ---- per-head pipeline: q/k projection -> scores -> softmax-over-l ->
        # attn@v -> contiguous copy into the scrambled proj input -> out proj.
        def qk_pair(h, part, base, dst, l, nl):
            # fp8 DoubleRow: 4 e-chunk-pairs of K=256, N = nl*256 moving
            p_qk = mm_ps.tile([128, nl * S], dt.float32, tag="mm", name="p_qk")
            q0, off = l // 2, (l % 2) * 256
            for p in range(NP):
                nc.tensor.matmul(
                    p_qk,
                    lhsT=wqk_sb[(part, h)][:, p, :, :],
                    rhs=x8_sb[p][:, q0, :, off:off + nl * S],
                    start=(p == 0),
                    stop=(p == NP - 1) and not with_bias,
                    perf_mode=DR,
                )
            if with_bias:
                nc.tensor.matmul(
                    p_qk,
                    lhsT=bq_sb[:, base + h * 128: base + (h + 1) * 128],
                    rhs=ones_sb[:, :nl * S],
                    start=False,
                    stop=True,
                )
            src = p_qk.rearrange("p (a b) -> p a b", a=nl)
            if part == "q":
                nc.scalar.copy(out=dst[:, l:l + nl, :], in_=src)
            else:
                nc.vector.tensor_copy(out=dst[:, l:l + nl, :], in_=src)

        def qk_proj(h):
            qT = qk_pool.tile([128, L, S], dt.float16, tag="qT", name="qT_sb")
            kT = qk_pool.tile([128, L, S], dt.float16, tag="kT", name="kT_sb")
            for part, base, dst in (("q", 0, qT), ("k", E, kT)):
                l = h
                if l % 2 == 1:  # x8 is quarter-major: pairs must be even-aligned
                    qk_pair(h, part, base, dst, l, 1)
                    l += 1
                while l < L:
                    qk_pair(h, part, base, dst, l, 2)
                    l += 2
            return qT, kT

        # ---- HAM warm-up: dummy matmuls with no DMA deps fill the input
        # lead-in and open the PE clock gate before real work arrives.
        p_warm = mm_ps.tile([128, 512], dt.float32, tag="mm", name="p_warm")
        for i in range(8):
            nc.tensor.matmul(p_warm, lhsT=warm_w, rhs=warm_x,
                             start=(i == 0), stop=(i == 7))

        def scores_exp(h, l, qT_sb, kT_sb, E_sb):
            p_sc = sc_ps.tile([128, 2, S], dt.float32, tag="sc", name="p_sc")
            for tc2 in range(2):
                nc.tensor.matmul(
                    p_sc[:, tc2, :],
                    lhsT=kT_sb[:, l, tc2 * 128:(tc2 + 1) * 128],
                    rhs=qT_sb[:, l, :],
                    start=True,
                    stop=True,
                )
            idx = l * H + h
            nc.scalar.activation(
                out=E_sb[:, l, :, :],
                in_=p_sc,
                func=AF.Exp,
                scale=dec_sb[:, idx:idx + 1],
            )

        # ---- front: head-0 q/k pairs, v-groups AND head-0 scores/exps
        # interleaved by xT quarter, matching DMA arrival order so the
        # in-order PE never blocks on a not-yet-loaded chunk. Head-0's
        # softmax runs on ACT/Pool underneath the v-projection.
        qT0 = qk_pool.tile([128, L, S], dt.float16, tag="qT", name="qT_sb")
        kT0 = qk_pool.tile([128, L, S], dt.float16, tag="kT", name="kT_sb")
        E0_sb = exp_pool.tile([128, L, 2, S], dt.float16, tag="E", name="E_sb")
        P0_sb = sm_pool.tile([128, 4, 2, S], dt.float16, tag="P", name="P_sb")
        for p in range(4):
            qk_pair(0, "q", 0, qT0, 2 * p, 2)
            qk_pair(0, "k", E, kT0, 2 * p, 2)
            v_proj(range(4 * p, 4 * p + 2))
            scores_exp(0, 2 * p, qT0, kT0, E0_sb)
            scores_exp(0, 2 * p + 1, qT0, kT0, E0_sb)
            nc.gpsimd.tensor_add(
                P0_sb[:, p], E0_sb[:, 2 * p], E0_sb[:, 2 * p + 1]
            )
            v_proj(range(4 * p + 2, 4 * p + 4))

        # packed-GT out projection: head h's nonzero rows live at pack offset
        # GOFF[h]; 128-row matmul blocks span head boundaries.
        GOFF = [0]
        for h in range(H):
            GOFF.append(GOFF[-1] + 256 - 32 * h)  # [0,256,480,...,1120,1152]
        # block b is ready once every head overlapping it has run attn@v
        # (head 7 runs in the front, so it never gates)
        BLK_READY = [0, 0, 1, 2, 2, 3, 4, 5, 6]

        def emit_blk(b):
            for ng in range(2):
                p_pr = mm_ps.tile([128, 512], dt.float32, tag="mm", name="p_pr")
                for j in range(NE):
                    nc.tensor.matmul(
                        p_pr,
                        lhsT=gtm_sb[:, j, 128 * b:128 * (b + 1)],
                        rhs=wo_sb[j][:, ng * 512:(ng + 1) * 512],
                        start=(j == 0),
                        stop=(j == NE - 1) and not with_bias,
                    )
                if with_bias:
                    nc.tensor.matmul(
                        p_pr,
                        lhsT=ones_sb[:, :128],
                        rhs=bo_sb[:, ng * 512:(ng + 1) * 512],
                        start=False,
                        stop=True,
                    )
                o_sb = out_pool.tile([128, 512], dt.float16, tag="o", name="o_sb")
                nc.vector.tensor_copy(out=o_sb, in_=p_pr)
                for h in range(H):
                    p0 = max(GOFF[h], 128 * b)
                    p1 = min(GOFF[h + 1], 128 * (b + 1))
                    if p0 >= p1:
                        continue
                    s0 = 32 * h + (p0 - GOFF[h])
                    nc.gpsimd.dma_start(
                        out=y_d[h, s0:s0 + (p1 - p0), ng * 512:(ng + 1) * 512],
                        in_=o_sb[p0 - 128 * b:p1 - 128 * b, :],
                    )

        # head 7 has a single unmasked layer: attn == 1 exactly, so attn@v is
        # a plain row-sum of v -- no scores/softmax at all. Run it right after
        # the front.
        p_o27 = o2_ps.tile([128, 2, S], dt.float32, tag="o2", name="p_o2")
        for tc2 in range(2):
            nc.tensor.matmul(
                p_o27[:, 0, :],
                lhsT=v_sb[:, VBLK[(7, 7)], tc2, :],
                rhs=ones_t,
                start=(tc2 == 0),
                stop=(tc2 == 1),
            )
        nc.vector.tensor_copy(
            out=gtm_sb[:, :, GOFF[7]:GOFF[8]],
            in_=p_o27[:, 0, :].rearrange("p (j si) -> p j si", j=8),
        )

        qk_tiles = {1: qk_proj(1)}  # depth-2 pipeline: projections 2 heads ahead
        for h in range(H - 1):

            ls = list(range(h, L))
            n = len(ls)
            if h == 0:
                # scores/exps/pair-sums already ran inside the front
                E_sb, P_sb = E0_sb, P0_sb
            else:
                # scores (transposed [t, s]) + exp with decay/sqrt(d) folded
                # into the activation scale, then denominator pair-sums on the
                # idle Pool engine (short tail reduce on DVE below)
                E_sb = exp_pool.tile([128, L, 2, S], dt.float16, tag="E", name="E_sb")
                P_sb = sm_pool.tile([128, 4, 2, S], dt.float16, tag="P", name="P_sb")
                qT_sb, kT_sb = qk_tiles.pop(h)
                for l in ls:
                    scores_exp(h, l, qT_sb, kT_sb, E_sb)
                for i in range(n // 2):
                    nc.gpsimd.tensor_add(
                        P_sb[:, i], E_sb[:, ls[2 * i]], E_sb[:, ls[2 * i + 1]]
                    )
            terms = [P_sb[:, i] for i in range(n // 2)]
            if n % 2:
                terms.append(E_sb[:, ls[-1]])

            # next heads' projection + ready out-projection blocks fill the
            # PE while this head's softmax chain finishes on ACT/DVE/Pool
            if h + 2 < H - 1:  # head 7 needs no q/k (attn == 1 shortcut)
                qk_tiles[h + 2] = qk_proj(h + 2)
            for b in range(9):
                if BLK_READY[b] == h - 1:
                    emit_blk(b)

            if len(terms) == 1:
                D_ap = terms[0]
            else:
                D_sb = sm_pool.tile([128, 2, S], dt.float16, tag="D", name="D_sb")
                nc.vector.tensor_add(D_sb, terms[0], terms[1])
                for t in terms[2:]:
                    nc.vector.tensor_add(D_sb, D_sb, t)
                D_ap = D_sb

            # 1/D: one fast custom-DVE op (fp32), fp16 casts on ACT
            D32 = sm_pool.tile([128, 2, S], dt.float32, tag="D32", name="D32")
            U32 = sm_pool.tile([128, 2, S], dt.float32, tag="U32", name="U32")
            U_sb = sm_pool.tile([128, 2, S], dt.float16, tag="U", name="U_sb")
            nc.scalar.copy(out=D32, in_=D_ap)
            nc.vector.reciprocal_approx_fast(out=U32, in_=D32)
            nc.scalar.copy(out=U_sb, in_=U32)

            # attn @ v (output transposed [dd, s~]): the host token permutation
            # makes the moving axis order (j, si), so the copy into the packed
            # GT is contiguous. Two l's share one PSUM bank and one copy.
            li = 0
            while li < n:
                nl2 = 2 if li + 1 < n else 1
                p_o2 = o2_ps.tile([128, 2, S], dt.float32, tag="o2", name="p_o2")
                for i in range(nl2):
                    l = ls[li + i]
                    at_sb = at_pool.tile([128, 2, S], dt.float16, tag="at", name="at_sb")
                    nc.vector.tensor_mul(at_sb, E_sb[:, l, :, :], U_sb)
                    for tc2 in range(2):
                        nc.tensor.matmul(
                            p_o2[:, i, :],
                            lhsT=v_sb[:, VBLK[(l, h)], tc2, :],
                            rhs=at_sb[:, tc2, :],
                            start=(tc2 == 0),
                            stop=(tc2 == 1),
                        )
                c0 = GOFF[h] + (ls[li] - h) * 32
                if nl2 == 2:
                    nc.vector.tensor_copy(
                        out=gtm_sb[:, :, c0:c0 + 64].rearrange(
                            "p j (ll si) -> p ll j si", ll=2),
                        in_=p_o2.rearrange("p ll (j si) -> p ll j si", j=8),
                    )
                else:
                    nc.vector.tensor_copy(
                        out=gtm_sb[:, :, c0:c0 + 32],
                        in_=p_o2[:, 0, :].rearrange("p (j si) -> p j si", j=8),
                    )
                li += nl2

        for b in range(9):
            if BLK_READY[b] == 6:
                emit_blk(b)

    nc.compile()
    return nc


def _get_nc(with_bias):
    if with_bias not in _BUILD_CACHE:
        _BUILD_CACHE[with_bias] = _build(with_bias)
    return _BUILD_CACHE[with_bias]


def _prepare_in_maps(x, w_qkv, b_qkv, w_out, b_out, decay_params):
    f16 = np.float16
    f8 = ml_dtypes.float8_e4m3
    with_bias = bool(np.any(b_qkv != 0) or np.any(b_out != 0))

    # q/k weights: [part, head, p, (pair, kk, m)] fp8
    wqk8 = np.ascontiguousarray(
        w_qkv[:2 * E].astype(f8)
        .reshape(2, H, d, NP, 2, 128)      # [part, h, m, pair, kk, p]
        .transpose(0, 1, 5, 3, 4, 2)       # [part, h, p, pair, kk, m]
    ).reshape(2, H, 128, E)
    wvT = np.ascontiguousarray(w_qkv[2 * E:].astype(f16).T)      # [E, E]
    woutT = np.ascontiguousarray(w_out.astype(f16).T)            # [E, E]

    in_maps = []
    for b in range(B):
        xp = x[b].reshape(L, S, E)[:, PERM, :].reshape(T, E)     # permuted tokens
        xT = np.ascontiguousarray(xp.astype(f16).T)              # [E, T]
        x8 = np.ascontiguousarray(
            xp.astype(f8).T                                      # [E, T]
            .reshape(NP, 2, 128, 4, 512)                         # [pair, kk, p, q, t']
            .transpose(0, 3, 2, 1, 4)                            # [pair, q, p, kk, t']
        )
        dec = np.ascontiguousarray(
            np.broadcast_to(
                (decay_params[b, :L, :H] / np.float32(np.sqrt(d)))
                .astype(np.float32)
                .reshape(1, L * H),
                (128, L * H),
            )
        )
        m = {"x8": x8, "xT": xT, "wqk8": wqk8, "wvT": wvT, "woutT": woutT,
             "decay": dec}
        if with_bias:
            m["bqkv"] = np.ascontiguousarray(b_qkv.astype(f16).reshape(1, F))
            m["bout"] = np.ascontiguousarray(b_out.astype(f16).reshape(1, E))
            m["bout_row"] = np.ascontiguousarray(
                np.broadcast_to(b_out.astype(f16).reshape(1, E), (128, E))
            )
        in_maps.append(m)
    return with_bias, in_maps


def _run(x, w_qkv, b_qkv, w_out, b_out, decay_params, **spmd_kwargs):
    from concourse.bass_utils import run_bass_kernel_spmd

    with_bias, in_maps = _prepare_in_maps(x, w_qkv, b_qkv, w_out, b_out, decay_params)
    nc = _get_nc(with_bias)
    res = run_bass_kernel_spmd(nc, in_maps, core_ids=list(range(B)), **spmd_kwargs)
    out = np.stack([r["y"] for r in res.results], axis=0)  # [B, H, S, E] fp16
    return out.astype(np.float32), res


def kernel(x, w_qkv, b_qkv, w_out, b_out, decay_params):
    out, _ = _run(
        np.asarray(x), np.asarray(w_qkv), np.asarray(b_qkv),
        np.asarray(w_out), np.asarray(b_out), np.asarray(decay_params),
    )
    return out


# revision 27
# speedup vs baseline: 1.0176x; 1.0176x over previous
"""Trainium2 Bass kernel for nn_CausalTemporalAttention.

Reference semantics (B == L == H == 8 required by the module's broadcast quirks):
  qkv = x @ w_qkv.T + b_qkv ; split q,k,v -> [B,L,H,S,d]
  scores[b,l,h,s,t] = q.k/sqrt(d) ; masked to -1e9 where h > l
  z = scores * decay_params[b,l,h] ; attn = softmax over l (the layer axis)
  out[b,l,h,s,:] = attn @ v ; swap (l,h) ; row-major reshape to [B*H, S, E]
  y = out @ w_out.T + b_out ; reshape [B,L,S,E]

Sharding: data-parallel over batch B across 8 cores (core i handles b=i).

Design:
  - q/k projections run in fp8e4 DoubleRow perf mode (K=256 per pass, 2x PE
    throughput); numerically validated at rel-err ~1.3e-2 vs the 2e-2 gate.
  - All other matmuls and on-chip intermediates are fp16 (same PE/DVE speed
    as bf16, 8x finer mantissa) so the fp8 stage gets the whole error budget.
  - Host permutes tokens within each layer (s~ = (s%8)*32 + s//8) so the
    attn@v output lands in the out-projection's scrambled (j, si) order and
    the PSUM->SBUF scatter becomes a contiguous copy.
  - Head 7 sees a single unmasked layer, so its attention weights are exactly
    1: attn@v collapses to a ones-matmul row sum and skips softmax entirely.
  - Out-projection only computes the nonzero rows s' >= 32h of each head's
    output; the zero rows are DMA-filled from a zero tile early on.
  - Softmax denominator: exp pair-sums on Pool, short reduce + one
    reciprocal_approx_fast (fp32 custom DVE op) on DVE, dtype casts on ACT.
  - x is loaded via two DMA queues in consumption order (the front is
    DMA-arrival bound otherwise); bulk weight loads sit on queues with no
    compute behind them.
"""

import os
import sys

import numpy as np
import ml_dtypes

if "/opt/trn_rl_repo" not in sys.path:
    sys.path.insert(0, "/opt/trn_rl_repo")

B, L, S, E = 8, 8, 256, 1024
H, d = 8, E // 8
T = L * S            # 2048 tokens per batch element
NE = E // 128        # 8 e-chunks
NP = NE // 2         # 4 fp8 DoubleRow e-chunk pairs
F = 3 * E

# token permutation within each layer: position p holds old token (p%32)*8+p//32
# so attn@v's moving axis comes out in the out-projection's (j=s%8, si=s//8)
# order and the gt scatter is contiguous.
PERM = np.array([(p % 32) * 8 + p // 32 for p in range(S)], dtype=np.int64)

# (l, h) pairs with h <= l, l-major so v-proj copies can batch whole h-groups
VBLK = {(l, h): l * (l + 1) // 2 + h for l in range(L) for h in range(l + 1)}
NVB = L * (L + 1) // 2  # 36 blocks

_BUILD_CACHE = {}


def _build(with_bias):
    import concourse.bass as bass
    import concourse.tile as tile
    import concourse.mybir as mybir
    from concourse import bacc
    from contextlib import ExitStack

    dt = mybir.dt
    AF = mybir.ActivationFunctionType
    DR = mybir.MatmulPerfMode.DoubleRow

    nc = bacc.Bacc("TRN2", target_bir_lowering=False, debug=False, num_devices=8)

    # fp8 x for q/k projection, quarter-major so one DMA lands one quarter with
    # 1KB-contiguous partition lines: [pair, q, p, kk, 512]
    x8_d = nc.dram_tensor("x8", [NP, 4, 128, 2, 512], dt.float8e4, kind="ExternalInput").ap()
    # fp16 x for the v projection (stationary side): [E, T]
    xT_d = nc.dram_tensor("xT", [E, T], dt.float16, kind="ExternalInput").ap()
    # q/k weights pre-packed as [part, head, p, (pair, kk, m)] so each
    # (part, head) is one contiguous [128, E] fp8 tile whose [:, pair, :, :]
    # slice is the DoubleRow stationary operand.
    wqk_d = nc.dram_tensor("wqk8", [2, H, 128, E], dt.float8e4, kind="ExternalInput").ap()
    wv_d = nc.dram_tensor("wvT", [E, E], dt.float16, kind="ExternalInput").ap()
    wo_d = nc.dram_tensor("woutT", [E, E], dt.float16, kind="ExternalInput").ap()
    dec_d = nc.dram_tensor("decay", [128, L * H], dt.float32, kind="ExternalInput").ap()
    if with_bias:
        bq_d = nc.dram_tensor("bqkv", [1, F], dt.float16, kind="ExternalInput").ap()
        bo_d = nc.dram_tensor("bout", [1, E], dt.float16, kind="ExternalInput").ap()
        bor_d = nc.dram_tensor("bout_row", [128, E], dt.float16, kind="ExternalInput").ap()
    y_d = nc.dram_tensor("y", [H, S, E], dt.float16, kind="ExternalOutput").ap()

    with ExitStack() as ctx:
        ctx.enter_context(
            nc.allow_low_precision(
                reason="fp8 q/k projection + fp16 softmax intermediates; "
                       "end-to-end error ~1.3e-2 of scale vs 2e-2 gate"
            )
        )
        tc = ctx.enter_context(tile.TileContext(nc))

        consts = ctx.enter_context(tc.tile_pool(name="consts", bufs=1))
        x8_sb = [consts.tile([128, 4, 2, 512], dt.float8e4, name=f"x8_{p}") for p in range(NP)]
        xT_sb = [consts.tile([128, T], dt.float16, name=f"xT{e}") for e in range(NE)]
        wqk_sb = {
            (part, h): consts.tile([128, NP, 2, 128], dt.float8e4, name=f"w{part}{h}")
            for part in ("q", "k")
            for h in range(H)
        }
        wv_sb = [consts.tile([128, E], dt.float16, name=f"wv{e}") for e in range(NE)]
        wo_sb = [consts.tile([128, E], dt.float16, name=f"wo{e}") for e in range(NE)]
        dec_sb = consts.tile([128, L * H], dt.float32)
        v_sb = consts.tile([128, NVB, 2, d], dt.float16)
        zrow_sb = consts.tile([128, 512], dt.float16)
        ones_t = consts.tile([128, S], dt.float16, name="ones_t")  # head-7 attn==1
        # all heads' nonzero out-proj rows packed into one [128, j, 1152] GT
        # so row-blocks can span head boundaries: 9 matmul blocks instead of 12
        gtm_sb = consts.tile([128, NE, 1152], dt.float16, name="gtm")

        if with_bias:
            bq_sb = consts.tile([1, F], dt.float16)
            bo_sb = consts.tile([1, E], dt.float16)
            ones_sb = consts.tile([1, 512], dt.float16)
            borow_sb = consts.tile([128, E], dt.float16)
            nc.gpsimd.dma_start(out=bq_sb, in_=bq_d)
            nc.gpsimd.dma_start(out=bo_sb, in_=bo_d)
            nc.gpsimd.dma_start(out=borow_sb, in_=bor_d)
            nc.gpsimd.memset(ones_sb, 1.0)

        # warm-up tiles first on the idle Pool engine so the PE clock-gate
        # opener isn't stuck behind DVE/ACT work
        warm_w = consts.tile([128, 128], dt.bfloat16, name="warm_w")
        warm_x = consts.tile([128, 512], dt.bfloat16, name="warm_x")
        nc.gpsimd.memset(warm_w, 0.0)
        nc.gpsimd.memset(warm_x, 0.0)
        nc.gpsimd.memset(zrow_sb, 0.0)
        nc.gpsimd.memset(ones_t, 1.0)
        nc.gpsimd.dma_start(out=dec_sb, in_=dec_d)

        # ---- DMA issue plan. Each dma_start lands on ONE ~25-40GB/s HW ring
        # (8 rings per issuing engine); the front consumes ~6MB of x in ~25us,
        # which saturates one queue's rings, so x is split across two queues in
        # exact consumption order. Queues that later run compute carry no DMA
        # tail (the bf16 baseline stalled head-0's exps behind weight DMAs).
        #   SP(sync): wqk8 h0 -> {x8 + xT16 lo-chunks per quarter} -> wqk8 h1
        #             -> wo16 -> zero-row y fills
        #   ACT:      {xT16 hi-chunks q0, wv g0, hi q1, wv g1, hi q2, hi q3}
        #             -> wqk8 h2..7   (all retired long before the first exp)
        def _wqk_dma(eng, pi, part, h):
            for half in range(2):
                eng.dma_start(
                    out=wqk_sb[(part, h)].rearrange("p a b c -> p (a b c)")[:, half * 512:(half + 1) * 512],
                    in_=wqk_d[pi, h, :, half * 512:(half + 1) * 512],
                )

        def _xT_dma(eng, e, q):
            eng.dma_start(
                out=xT_sb[e][:, q * 512:(q + 1) * 512],
                in_=xT_d[e * 128:(e + 1) * 128, q * 512:(q + 1) * 512],
            )

        # The first quarter's burst (wqk h0 + x8 q0 + xT q0, ~1.5MB) gates the
        # whole front; spread it over three queues' ring groups so the
        # transfers run in parallel instead of serializing on one queue.
        def _x8_halves(eng, p, q):
            for kk in range(2):
                eng.dma_start(out=x8_sb[p][:, q, kk, :], in_=x8_d[p, q, :, kk, :])

        _wqk_dma(nc.sync, 0, "q", 0)
        _x8_halves(nc.sync, 0, 0)
        _wqk_dma(nc.sync, 1, "k", 0)
        for e in range(2):
            _xT_dma(nc.sync, e, 0)
        for pi, part in ((0, "q"), (1, "k")):
            _wqk_dma(nc.sync, pi, part, 1)
        for q in range(1, 4):
            for p in range(NP):
                nc.sync.dma_start(out=x8_sb[p][:, q, :, :], in_=x8_d[p, q])
            for e in range(4):
                _xT_dma(nc.sync, e, q)
        # ACT queue carries only x chunks (done ~13us) so head-0's exps are
        # never stuck behind a DMA-issue tail; wv rides the Pool queue whose
        # compute (pair-sums) starts even later.
        _x8_halves(nc.scalar, 1, 0)
        _x8_halves(nc.scalar, 2, 0)
        for q in range(4):
            for e in range(4, NE):
                _xT_dma(nc.scalar, e, q)
            if q == 1:  # wv cols 512+ first needed by v_proj l=4 (~quarter 2)
                for e in range(NE):
                    nc.scalar.dma_start(
                        out=wv_sb[e][:, 512:], in_=wv_d[e * 128:(e + 1) * 128, 512:]
                    )
        _x8_halves(nc.gpsimd, 3, 0)
        for e in range(2, 4):
            _xT_dma(nc.gpsimd, e, 0)
        for e in range(NE):
            nc.gpsimd.dma_start(
                out=wv_sb[e][:, :512], in_=wv_d[e * 128:(e + 1) * 128, :512]
            )
        # late-needed weights + zero-row y fills on the sync tail, in
        # consumption order (h2.. weights first, zero fills last)
        for h in range(2, H):
            for pi, part in ((0, "q"), (1, "k")):
                _wqk_dma(nc.sync, pi, part, h)
        for e in range(NE):
            for half in range(2):
                nc.sync.dma_start(
                    out=wo_sb[e][:, half * 512:(half + 1) * 512],
                    in_=wo_d[e * 128:(e + 1) * 128, half * 512:(half + 1) * 512],
                )
        zsrc = borow_sb if with_bias else None
        for h in range(1, H):
            r = 32 * h
            for (r0, r1) in ((0, min(r, 128)), (128, r)):
                if r1 <= r0:
                    continue
                for ng in range(2):
                    src = (zsrc[:r1 - r0, ng * 512:(ng + 1) * 512] if with_bias
                           else zrow_sb[:r1 - r0, :])
                    nc.sync.dma_start(
                        out=y_d[h, r0:r1, ng * 512:(ng + 1) * 512], in_=src
                    )

        mm_ps = ctx.enter_context(tc.tile_pool(name="mm_ps", bufs=4, space="PSUM"))
        sc_ps = ctx.enter_context(tc.tile_pool(name="sc_ps", bufs=2, space="PSUM"))
        o2_ps = ctx.enter_context(tc.tile_pool(name="o2_ps", bufs=2, space="PSUM"))

        qk_pool = ctx.enter_context(tc.tile_pool(name="qk", bufs=3))
        exp_pool = ctx.enter_context(tc.tile_pool(name="expp", bufs=2))
        sm_pool = ctx.enter_context(tc.tile_pool(name="smp", bufs=2))
        at_pool = ctx.enter_context(tc.tile_pool(name="atp", bufs=3))
        out_pool = ctx.enter_context(tc.tile_pool(name="outp", bufs=3))

        def v_proj(tts, gs=(0, 1)):
            # v projection (natural [token, dd] layout): stationary xT chunk,
            # moving wv columns. Only heads h <= l are ever computed; copies
            # batch all h-blocks of one PSUM group (v_sb is l-major).
            for tt in tts:
                l = tt // 2
                ncols = 128 * (l + 1)
                for g in range((ncols + 511) // 512):
                    if g not in gs:
                        continue
                    n_g = min(512, ncols - 512 * g)
                    p_v = mm_ps.tile([128, n_g], dt.float32, tag="mm", name="p_v")
                    for e in range(NE):
                        nc.tensor.matmul(
                            p_v,
                            lhsT=xT_sb[e][:, tt * 128:(tt + 1) * 128],
                            rhs=wv_sb[e][:, 512 * g: 512 * g + n_g],
                            start=(e == 0),
                            stop=(e == NE - 1) and not with_bias,
                        )
                    if with_bias:
                        nc.tensor.matmul(
                            p_v,
                            lhsT=ones_sb[:, :128],
                            rhs=bq_sb[:, 2 * E + 512 * g: 2 * E + 512 * g + n_g],
                            start=False,
                            stop=True,
                        )
                    vb = VBLK[(l, 4 * g)]
                    nb = n_g // 128
                    nc.vector.tensor_copy(
                        out=v_sb[:, vb:vb + nb, tt % 2, :],
                        in_=p_v.rearrange("p (b m) -> p b m", b=nb),
                    )

        # ---- per-head pipeline: q/k projection -> scores -> softmax-over-l ->
        # attn@v -> contiguous copy into the scrambled proj input -> out proj.
        def qk_pair(h, part, base, dst, l, nl):
            # fp8 DoubleRow: 4 e-chunk-pairs of K=256, N = nl*256 moving
            p_qk = mm_ps.tile([128, nl * S], dt.float32, tag="mm", name="p_qk")
            q0, off = l // 2, (l % 2) * 256
            for p in range(NP):
                nc.tensor.matmul(
                    p_qk,
                    lhsT=wqk_sb[(part, h)][:, p, :, :],
                    rhs=x8_sb[p][:, q0, :, off:off + nl * S],
                    start=(p == 0),
                    stop=(p == NP - 1) and not with_bias,
                    perf_mode=DR,
                )
            if with_bias:
                nc.tensor.matmul(
                    p_qk,
                    lhsT=bq_sb[:, base + h * 128: base + (h + 1) * 128],
                    rhs=ones_sb[:, :nl * S],
                    start=False,
                    stop=True,
                )
            src = p_qk.rearrange("p (a b) -> p a b", a=nl)
            if part == "q":
                nc.scalar.copy(out=dst[:, l:l + nl, :], in_=src)
            else:
                nc.vector.tensor_copy(out=dst[:, l:l + nl, :], in_=src)

        def qk_proj(h):
            qT = qk_pool.tile([128, L, S], dt.float16, tag="qT", name="qT_sb")
            kT = qk_pool.tile([128, L, S], dt.float16, tag="kT", name="kT_sb")
            for part, base, dst in (("q", 0, qT), ("k", E, kT)):
                l = h
                if l % 2 == 1:  # x8 is quarter-major: pairs must be even-aligned
                    qk_pair(h, part, base, dst, l, 1)
                    l += 1
                while l < L:
                    qk_pair(h, part, base, dst, l, 2)
                    l += 2
            return qT, kT

        # ---- HAM warm-up: dummy matmuls with no DMA deps fill the input
        # lead-in and open the PE clock gate before real work arrives.
        p_warm = mm_ps.tile([128, 512], dt.float32, tag="mm", name="p_warm")
        for i in range(8):
            nc.tensor.matmul(p_warm, lhsT=warm_w, rhs=warm_x,
                             start=(i == 0), stop=(i == 7))

        def scores_exp(h, l, qT_sb, kT_sb, E_sb):
            p_sc = sc_ps.tile([128, 2, S], dt.float32, tag="sc", name="p_sc")
            for tc2 in range(2):
                nc.tensor.matmul(
                    p_sc[:, tc2, :],
                    lhsT=kT_sb[:, l, tc2 * 128:(tc2 + 1) * 128],
                    rhs=qT_sb[:, l, :],
                    start=True,
                    stop=True,
                )
            idx = l * H + h
            nc.scalar.activation(
                out=E_sb[:, l, :, :],
                in_=p_sc,
                func=AF.Exp,
                scale=dec_sb[:, idx:idx + 1],
            )

        # ---- front: head-0 q/k pairs, v-groups AND head-0 scores/exps
        # interleaved by xT quarter, matching DMA arrival order so the
        # in-order PE never blocks on a not-yet-loaded chunk. Head-0's
        # softmax runs on ACT/Pool underneath the v-projection.
        qT0 = qk_pool.tile([128, L, S], dt.float16, tag="qT", name="qT_sb")
        kT0 = qk_pool.tile([128, L, S], dt.float16, tag="kT", name="kT_sb")
        E0_sb = exp_pool.tile([128, L, 2, S], dt.float16, tag="E", name="E_sb")
        P0_sb = sm_pool.tile([128, 4, 2, S], dt.float16, tag="P", name="P_sb")
        for p in range(4):
            qk_pair(0, "q", 0, qT0, 2 * p, 2)
            qk_pair(0, "k", E, kT0, 2 * p, 2)
            v_proj(range(4 * p, 4 * p + 2))
            scores_exp(0, 2 * p, qT0, kT0, E0_sb)
            scores_exp(0, 2 * p + 1, qT0, kT0, E0_sb)
            nc.gpsimd.tensor_add(
                P0_sb[:, p], E0_sb[:, 2 * p], E0_sb[:, 2 * p + 1]
            )
            v_proj(range(4 * p + 2, 4 * p + 4))

        # packed-GT out projection: head h's nonzero rows live at pack offset
        # GOFF[h]; 128-row matmul blocks span head boundaries.
        GOFF = [0]
        for h in range(H):
            GOFF.append(GOFF[-1] + 256 - 32 * h)  # [0,256,480,...,1120,1152]
        # block b is ready once every head overlapping it has run attn@v
        # (head 7 runs in the front, so it never gates). Emission is pushed
        # later than readiness for the late heads, whose qk-projection filler
        # has run out by then.
        BLK_EMIT = {1: [0], 2: [1], 3: [2], 4: [3], 5: [4, 5], 6: [6, 7], 7: [8]}

        def emit_blk(b):
            for ng in range(2):
                p_pr = mm_ps.tile([128, 512], dt.float32, tag="mm", name="p_pr")
                for j in range(NE):
                    nc.tensor.matmul(
                        p_pr,
                        lhsT=gtm_sb[:, j, 128 * b:128 * (b + 1)],
                        rhs=wo_sb[j][:, ng * 512:(ng + 1) * 512],
                        start=(j == 0),
                        stop=(j == NE - 1) and not with_bias,
                    )
                if with_bias:
                    nc.tensor.matmul(
                        p_pr,
                        lhsT=ones_sb[:, :128],
                        rhs=bo_sb[:, ng * 512:(ng + 1) * 512],
                        start=False,
                        stop=True,
                    )
                o_sb = out_pool.tile([128, 512], dt.float16, tag="o", name="o_sb")
                nc.vector.tensor_copy(out=o_sb, in_=p_pr)
                for h in range(H):
                    p0 = max(GOFF[h], 128 * b)
                    p1 = min(GOFF[h + 1], 128 * (b + 1))
                    if p0 >= p1:
                        continue
                    s0 = 32 * h + (p0 - GOFF[h])
                    nc.gpsimd.dma_start(
                        out=y_d[h, s0:s0 + (p1 - p0), ng * 512:(ng + 1) * 512],
                        in_=o_sb[p0 - 128 * b:p1 - 128 * b, :],
                    )

        # head 7 has a single unmasked layer: attn == 1 exactly, so attn@v is
        # a plain row-sum of v -- no scores/softmax at all. Run it right after
        # the front.
        p_o27 = o2_ps.tile([128, 2, S], dt.float32, tag="o2", name="p_o2")
        for tc2 in range(2):
            nc.tensor.matmul(
                p_o27[:, 0, :],
                lhsT=v_sb[:, VBLK[(7, 7)], tc2, :],
                rhs=ones_t,
                start=(tc2 == 0),
                stop=(tc2 == 1),
            )
        nc.vector.tensor_copy(
            out=gtm_sb[:, :, GOFF[7]:GOFF[8]],
            in_=p_o27[:, 0, :].rearrange("p (j si) -> p j si", j=8),
        )

        qk_tiles = {1: qk_proj(1)}  # depth-2 pipeline: projections 2 heads ahead
        for h in range(H - 1):

            ls = list(range(h, L))
            n = len(ls)
            if h == 0:
                # scores/exps/pair-sums already ran inside the front
                E_sb, P_sb = E0_sb, P0_sb
            else:
                # scores (transposed [t, s]) + exp with decay/sqrt(d) folded
                # into the activation scale, then denominator pair-sums on the
                # idle Pool engine (short tail reduce on DVE below)
                E_sb = exp_pool.tile([128, L, 2, S], dt.float16, tag="E", name="E_sb")
                P_sb = sm_pool.tile([128, 4, 2, S], dt.float16, tag="P", name="P_sb")
                qT_sb, kT_sb = qk_tiles.pop(h)
                for l in ls:
                    scores_exp(h, l, qT_sb, kT_sb, E_sb)
                for i in range(n // 2):
                    nc.gpsimd.tensor_add(
                        P_sb[:, i], E_sb[:, ls[2 * i]], E_sb[:, ls[2 * i + 1]]
                    )
            terms = [P_sb[:, i] for i in range(n // 2)]
            if n % 2:
                terms.append(E_sb[:, ls[-1]])

            # next heads' projection + ready out-projection blocks fill the
            # PE while this head's softmax chain finishes on ACT/DVE/Pool
            if h + 2 < H - 1:  # head 7 needs no q/k (attn == 1 shortcut)
                qk_tiles[h + 2] = qk_proj(h + 2)
            for b in BLK_EMIT.get(h, []):
                emit_blk(b)

            if len(terms) == 1:
                D_ap = terms[0]
            else:
                D_sb = sm_pool.tile([128, 2, S], dt.float16, tag="D", name="D_sb")
                nc.vector.tensor_add(D_sb, terms[0], terms[1])
                for t in terms[2:]:
                    nc.vector.tensor_add(D_sb, D_sb, t)
                D_ap = D_sb

            # 1/D: one fast custom-DVE op (fp32), fp16 casts on ACT
            D32 = sm_pool.tile([128, 2, S], dt.float32, tag="D32", name="D32")
            U32 = sm_pool.tile([128, 2, S], dt.float32, tag="U32", name="U32")
            U_sb = sm_pool.tile([128, 2, S], dt.float16, tag="U", name="U_sb")
            nc.scalar.copy(out=D32, in_=D_ap)
            nc.vector.reciprocal_approx_fast(out=U32, in_=D32)
            nc.scalar.copy(out=U_sb, in_=U32)

            # attn @ v (output transposed [dd, s~]): the host token permutation
            # makes the moving axis order (j, si), so the copy into the packed
            # GT is contiguous. Two l's share one PSUM bank and one copy.
            li = 0
            while li < n:
                nl2 = 2 if li + 1 < n else 1
                p_o2 = o2_ps.tile([128, 2, S], dt.float32, tag="o2", name="p_o2")
                for i in range(nl2):
                    l = ls[li + i]
                    at_sb = at_pool.tile([128, 2, S], dt.float16, tag="at", name="at_sb")
                    nc.vector.tensor_mul(at_sb, E_sb[:, l, :, :], U_sb)
                    for tc2 in range(2):
                        nc.tensor.matmul(
                            p_o2[:, i, :],
                            lhsT=v_sb[:, VBLK[(l, h)], tc2, :],
                            rhs=at_sb[:, tc2, :],
                            start=(tc2 == 0),
                            stop=(tc2 == 1),
                        )
                c0 = GOFF[h] + (ls[li] - h) * 32
                if nl2 == 2:
                    nc.vector.tensor_copy(
                        out=gtm_sb[:, :, c0:c0 + 64].rearrange(
                            "p j (ll si) -> p ll j si", ll=2),
                        in_=p_o2.rearrange("p ll (j si) -> p ll j si", j=8),
                    )
                else:
                    nc.vector.tensor_copy(
                        out=gtm_sb[:, :, c0:c0 + 32],
                        in_=p_o2[:, 0, :].rearrange("p (j si) -> p j si", j=8),
                    )
                li += nl2

        for b in BLK_EMIT[7]:
            emit_blk(b)

    nc.compile()
    return nc


def _get_nc(with_bias):
    if with_bias not in _BUILD_CACHE:
        _BUILD_CACHE[with_bias] = _build(with_bias)
    return _BUILD_CACHE[with_bias]


def _prepare_in_maps(x, w_qkv, b_qkv, w_out, b_out, decay_params):
    f16 = np.float16
    f8 = ml_dtypes.float8_e4m3
    with_bias = bool(np.any(b_qkv != 0) or np.any(b_out != 0))

    # q/k weights: [part, head, p, (pair, kk, m)] fp8
    wqk8 = np.ascontiguousarray(
        w_qkv[:2 * E].astype(f8)
        .reshape(2, H, d, NP, 2, 128)      # [part, h, m, pair, kk, p]
        .transpose(0, 1, 5, 3, 4, 2)       # [part, h, p, pair, kk, m]
    ).reshape(2, H, 128, E)
    wvT = np.ascontiguousarray(w_qkv[2 * E:].astype(f16).T)      # [E, E]
    woutT = np.ascontiguousarray(w_out.astype(f16).T)            # [E, E]

    in_maps = []
    for b in range(B):
        xp = x[b].reshape(L, S, E)[:, PERM, :].reshape(T, E)     # permuted tokens
        xT = np.ascontiguousarray(xp.astype(f16).T)              # [E, T]
        x8 = np.ascontiguousarray(
            xp.astype(f8).T                                      # [E, T]
            .reshape(NP, 2, 128, 4, 512)                         # [pair, kk, p, q, t']
            .transpose(0, 3, 2, 1, 4)                            # [pair, q, p, kk, t']
        )
        dec = np.ascontiguousarray(
            np.broadcast_to(
                (decay_params[b, :L, :H] / np.float32(np.sqrt(d)))
                .astype(np.float32)
                .reshape(1, L * H),
                (128, L * H),
            )
        )
        m = {"x8": x8, "xT": xT, "wqk8": wqk8, "wvT": wvT, "woutT": woutT,
             "decay": dec}
        if with_bias:
            m["bqkv"] = np.ascontiguousarray(b_qkv.astype(f16).reshape(1, F))
            m["bout"] = np.ascontiguousarray(b_out.astype(f16).reshape(1, E))
            m["bout_row"] = np.ascontiguousarray(
                np.broadcast_to(b_out.astype(f16).reshape(1, E), (128, E))
            )
        in_maps.append(m)
    return with_bias, in_maps


def _run(x, w_qkv, b_qkv, w_out, b_out, decay_params, **spmd_kwargs):
    from concourse.bass_utils import run_bass_kernel_spmd

    with_bias, in_maps = _prepare_in_maps(x, w_qkv, b_qkv, w_out, b_out, decay_params)
    nc = _get_nc(with_bias)
    res = run_bass_kernel_spmd(nc, in_maps, core_ids=list(range(B)), **spmd_kwargs)
    out = np.stack([r["y"] for r in res.results], axis=0)  # [B, H, S, E] fp16
    return out.astype(np.float32), res


def kernel(x, w_qkv, b_qkv, w_out, b_out, decay_params):
    out, _ = _run(
        np.asarray(x), np.asarray(w_qkv), np.asarray(b_qkv),
        np.asarray(w_out), np.asarray(b_out), np.asarray(decay_params),
    )
    return out
